# revision 45
# baseline (speedup 1.0000x reference)
"""BalancedPrototypeLoss on 8 Trainium2 NeuronCores.

Strategy (data-parallel over batch, row-parallel over prototypes):
  - similarities [16384,100,10] sharded along batch across 8 cores
    (2048 samples/core = 16 tiles of 128 partitions), streamed in 4
    groups of 4 tiles with the P-dim split 5+5: the second half is
    DMA'd with a SWDGE accumulate-max onto the first (the DMA engine
    does tree level 1), the rest of the max-over-P tree is 3 dense
    fp16 tensor_tensor max ops (DVE 2x mode).
  - one-hot(-4x) generated on device from labels via iota+tensor_scalar.
  - per-tile own/other class stats via tensor_tensor_reduce
    (own = dot(smax, -4*onehot) row-sum; other = max(smax - 4*onehot)),
    then per-tile PE matmuls RHS^T @ (-4*onehot) accumulated in PSUM
    giving per-class partials [3,100].
  - prototypes are normalized + transposed + rotated by 125*core on the
    host (O(T*D) prep); gram rows for this core's 125-row slice via two
    PE matmuls into PSUM; diversity needs only a 146-col band around
    the diagonal (classes are 10-wide contiguous), contrastive row sums
    via scalar-engine copy-accumulate, diagonal via identity mask.
  - host combines the tiny per-core partials ([3,100] + [128,4] each)
    and evaluates the final scalar formulas in float32.
"""

import sys

_TRN_REPO = "/opt/trn_rl_repo"
if _TRN_REPO not in sys.path:
    sys.path.insert(0, _TRN_REPO)

import numpy as np

import concourse.bacc as bacc
import concourse.mybir as mybir
from concourse import tile
from concourse.masks import make_identity
from concourse.bass_utils import run_bass_kernel_spmd

fp32 = mybir.dt.float32
fp16 = mybir.dt.float16
Alu = mybir.AluOpType
Act = mybir.ActivationFunctionType
Axis = mybir.AxisListType

B, C, P, D, T = 16384, 100, 10, 256, 1000
NCORES = 8
BC = B // NCORES     # 2048 samples per core
NT = BC // 128       # 16 batch tiles per core
GRP = 4              # tiles per group
NG = NT // GRP       # 4 groups
TRV = T // NCORES    # 125 prototype rows per core
BAND0 = 137          # gram band: local cols [0,137)
BAND1 = 9            # plus wraparound cols [991,1000)
BANDW = BAND0 + BAND1
MARGIN = 0.3
CLST_SCALE = 0.8
SEP_SCALE = 0.08
DIV_SCALE = 0.01
CONTRASTIVE_SCALE = 0.1

USE_DMA_L1 = False   # neuronx-cc rejects DMA accum max (add-only CCE)
GROUPS = [2, 4, 4, 4, 2]          # batch tiles per streamed group
GOFF = [sum(GROUPS[:i]) for i in range(len(GROUPS))]
JO_ON_GPSIMD = False              # Q7 TT also slower + port contention
OH_ON_GPSIMD = False              # Q7 tensor_scalar is ~7x slower than DVE

_PROGRAMS = {}


def _build(dma_l1: bool):
    nc = bacc.Bacc("TRN2", target_bir_lowering=False, debug=False,
                   num_devices=1)
    sims_d = nc.dram_tensor("sims", [128, P * NT * C], fp16,
                            kind="ExternalInput").ap()
    lab_d = nc.dram_tensor("lab", [128, NT], fp32, kind="ExternalInput").ap()
    pt0_d = nc.dram_tensor("pt0", [128, T], fp16, kind="ExternalInput").ap()
    pt1_d = nc.dram_tensor("pt1", [128, T], fp16, kind="ExternalInput").ap()
    bandm_d = nc.dram_tensor("bandm", [128, BANDW], fp16,
                             kind="ExternalInput").ap()
    outcls_d = nc.dram_tensor("out_cls", [2, C], fp32, kind="ExternalOutput").ap()
    outown_d = nc.dram_tensor("out_own", [C, C], fp32, kind="ExternalOutput").ap()
    outpr_d = nc.dram_tensor("out_pr", [128, 4], fp32, kind="ExternalOutput").ap()

    with tile.TileContext(nc) as tc:
        with (
            tc.tile_pool(name="consts", bufs=1) as consts,
            tc.tile_pool(name="simin", bufs=2) as simin,
            tc.tile_pool(name="tree", bufs=2) as treep,
            tc.tile_pool(name="junk", bufs=2) as junkp,
            tc.tile_pool(name="outp", bufs=1) as outp,
            tc.tile_pool(name="psA", bufs=1, space="PSUM") as psA,
            tc.tile_pool(name="psG", bufs=2, space="PSUM") as psG,
        ):
            # ---- hoisted loads: first sims group leads, consts interleave ----
            sAs = []
            for g, gsz in enumerate(GROUPS):
                sAs.append(consts.tile([128, P, gsz, C], fp16, name=f"sA{g}",
                                       tag=f"sA{g}"))
            lab = consts.tile([128, NT], fp32, tag="lab")
            pt0 = consts.tile([128, T], fp16, tag="pt0")
            pt1 = consts.tile([128, T], fp16, tag="pt1")
            bandm = consts.tile([128, BANDW], fp16, tag="bandm")

            def sims_dma(g, eng):
                off = P * GOFF[g] * C
                sz = P * GROUPS[g] * C
                eng.dma_start(sAs[g][:], sims_d[:, off:off + sz])

            nc.sync.dma_start(lab[:], lab_d[:])
            for g in range(len(GROUPS)):
                sims_dma(g, nc.sync)
            nc.sync.dma_start(pt0[:], pt0_d[:])
            nc.sync.dma_start(pt1[:], pt1_d[:])
            nc.sync.dma_start(bandm[:], bandm_d[:])

            iot = consts.tile([128, C], fp16, tag="iot")
            nc.gpsimd.iota(iot[:], pattern=[[1, C]], base=0,
                           channel_multiplier=0,
                           allow_small_or_imprecise_dtypes=True)
            nhalf = consts.tile([128, 1], fp32, tag="nhalf")
            nc.vector.memset(nhalf[:], -0.5)

            # one-hot * (-4) for every tile, on device from labels
            oh_eng = nc.gpsimd if OH_ON_GPSIMD else nc.vector
            OH = consts.tile([128, NT, C], fp16, tag="OH")
            with tc.high_priority():
                for i in range(NT):
                    oh_eng.tensor_scalar(OH[:, i, :], iot[:], lab[:, i:i + 1],
                                         -4.0, op0=Alu.is_equal, op1=Alu.mult)

            RHS = consts.tile([128, 2, NT], fp16, tag="RHS")
            nc.vector.memset(RHS[:, 1, :], 1.0)
            maxr = consts.tile([128, NT], fp32, tag="maxr")

            # ---- gram: rows 0:128 (this core's rotated slice) ----
            g01 = []
            for nh in range(2):
                g = psG.tile([128, 500], fp32, name=f"g{nh}", tag=f"g{nh}")
                nc.tensor.matmul(g[:], pt0[:, 0:128], pt0[:, 500 * nh:500 * (nh + 1)],
                                 start=True, stop=False)
                nc.tensor.matmul(g[:], pt1[:, 0:128], pt1[:, 500 * nh:500 * (nh + 1)],
                                 start=False, stop=True)
                g01.append(g)

            opr = outp.tile([128, 4], fp32, tag="opr")
            # diversity band: relu(g - 0.5) over local cols [0,137) u [991,1000)
            bandr = junkp.tile([128, BANDW], fp16, tag="bandr")
            nc.scalar.activation(bandr[:, 0:BAND0], g01[0][:, 0:BAND0],
                                 Act.Relu, bias=nhalf[:])
            nc.scalar.activation(bandr[:, BAND0:BANDW], g01[1][:, 491:500],
                                 Act.Relu, bias=nhalf[:])
            bandj = junkp.tile([128, BANDW], fp32, tag="bandj")
            nc.vector.tensor_tensor(bandj[:], bandr[:], bandm[:], op=Alu.mult)
            nc.vector.tensor_reduce(opr[:, 0:1], bandj[:], axis=Axis.X, op=Alu.add)
            # contrastive: full row sums (scalar engine accumulate)
            for nh in range(2):
                junkc = junkp.tile([128, 500], fp32, name=f"jc{nh}", tag="jc")
                nc.scalar.activation(junkc[:], g01[nh][:], Act.Copy,
                                     accum_out=opr[:, 1 + nh:2 + nh])
            # (diagonal self-similarity is reproduced on the host from pn16)
            nc.vector.memset(opr[:, 3:4], 0.0)
            nc.sync.dma_start(outpr_d[:], opr[:])

            # ---- batch stream ----
            cls_ps = psA.tile([2, C], fp32, tag="cls")
            own_ps = psA.tile([C, C], fp32, tag="own")
            for g, gsz in enumerate(GROUPS):
                t0 = GOFF[g]
                sl = slice(t0, t0 + gsz)
                last = (g == len(GROUPS) - 1)
                sA = sAs[g]
                t1 = treep.tile([128, 5, gsz, C], fp16, name=f"t1_{g}", tag="t1")
                nc.vector.tensor_tensor(t1[:], sA[:, 0:5], sA[:, 5:10],
                                        op=Alu.max)
                t2 = treep.tile([128, 2, gsz, C], fp16, name=f"t2_{g}", tag="t2")
                nc.vector.tensor_tensor(t2[:], t1[:, 0:2], t1[:, 2:4],
                                        op=Alu.max)
                t3 = treep.tile([128, 1, gsz, C], fp16, name=f"t3_{g}", tag="t3")
                nc.vector.tensor_tensor(t3[:], t2[:, 0:1], t2[:, 1:2], op=Alu.max)
                SM = treep.tile([128, gsz, C], fp16, name=f"SM{g}", tag="SM")
                nc.vector.tensor_tensor(SM[:], t3[:, 0], t1[:, 4], op=Alu.max)

                js = junkp.tile([128, gsz, C], fp16, name=f"js{g}", tag="js")
                # own: diag(OH^T @ SM) on the PE gives -4*per-class own sums
                for t in range(gsz):
                    i = t0 + t
                    nc.tensor.matmul(own_ps[:], OH[:, i, :], SM[:, t, :],
                                     start=(i == 0), stop=(i == NT - 1))
                # other: max(smax - 4*onehot)
                nc.vector.tensor_tensor(js[:], SM[:], OH[:, sl, :], op=Alu.add)
                nc.vector.tensor_reduce(maxr[:, sl], js[:], axis=Axis.X,
                                        op=Alu.max)
                # RHS rows: relu(other_smax - (1-margin)) ; 1
                nc.vector.tensor_scalar(RHS[:, 0, sl], maxr[:, sl],
                                        -(1.0 - MARGIN), 0.0,
                                        op0=Alu.add, op1=Alu.max)
                for t in range(gsz):
                    i = t0 + t
                    nc.tensor.matmul(cls_ps[:], RHS[:, :, i], OH[:, i, :],
                                     start=(i == 0), stop=(i == NT - 1))

            ocl = outp.tile([2, C], fp32, tag="ocl")
            nc.vector.tensor_copy(ocl[:], cls_ps[:])
            nc.sync.dma_start(outcls_d[:], ocl[:])
            oow = outp.tile([C, C], fp32, tag="oow")
            nc.vector.tensor_copy(oow[:], own_ps[:])
            nc.sync.dma_start(outown_d[:], oow[:])

    nc.compile()
    return nc


def _get_program():
    key = bool(USE_DMA_L1)
    if key not in _PROGRAMS:
        _PROGRAMS[key] = _build(key)
    return _PROGRAMS[key]


def _numpy_fallback(similarities, labels, prototypes, proto_indices, valid_mask):
    """Pure-numpy replication of the reference (for unexpected shapes)."""
    s = similarities.astype(np.float64)
    Bx, Cx, Px = s.shape
    Tx = prototypes.shape[0]
    distances = 1.0 - s
    starts = proto_indices[:, 0]
    ends = proto_indices[:, 1]
    counts = ends - starts
    pvalid = np.arange(Px)[None, :] < counts[:, None]
    dmask = np.where(pvalid[None, :, :], distances, np.inf)
    min_all = dmask.min(axis=-1)
    own_min = min_all[np.arange(Bx), labels]
    cls_n = np.bincount(labels, minlength=Cx).astype(np.float64)
    cls_sum = np.bincount(labels, weights=own_min, minlength=Cx)
    has = cls_n > 0
    nvalid = max(int(has.sum()), 1)
    mean_c = cls_sum / np.maximum(cls_n, 1.0)
    w = 1.0 / np.sqrt(cls_n + 1e-6)
    cluster = np.where(has, w * mean_c, 0.0).sum() / nvalid * CLST_SCALE
    m2 = min_all.copy()
    m2[np.arange(Bx), labels] = np.inf
    other_min = m2.min(axis=-1)
    sep_term = np.maximum(MARGIN - other_min, 0.0)
    sep_cls = np.bincount(labels, weights=sep_term, minlength=Cx)
    sep = np.where(has, sep_cls / np.maximum(cls_n, 1.0), 0.0).sum() / nvalid * SEP_SCALE
    pr = prototypes.astype(np.float64)
    norm = np.sqrt((pr * pr).sum(-1, keepdims=True))
    pn = pr / np.maximum(norm, 1e-12)
    sim = pn @ pn.T
    proto_class = np.searchsorted(starts, np.arange(Tx), side="right") - 1
    same = proto_class[:, None] == proto_class[None, :]
    offd = ~np.eye(Tx, dtype=bool)
    pair = same & offd
    relv = np.maximum(sim - 0.5, 0.0)
    row_sum = np.where(pair, relv, 0.0).sum(1)
    cls_pair = np.bincount(proto_class, weights=row_sum, minlength=Cx)
    npairs = (counts * (counts - 1)).astype(np.float64)
    dvalid = counts > 1
    ndv = max(int(dvalid.sum()), 1)
    div = np.where(dvalid, cls_pair / np.maximum(npairs, 1.0), 0.0).sum() / ndv * DIV_SCALE
    vm = valid_mask.astype(bool)
    vpair = (vm[:, None] & vm[None, :]) & offd
    nvp = max(int(vpair.sum()), 1)
    contrast = np.where(vpair, sim, 0.0).sum() / nvp * CONTRASTIVE_SCALE
    total = cluster + sep + div + contrast
    return np.array([cluster, sep, div, contrast, total], dtype=np.float32)


def kernel(similarities, labels, prototypes, proto_indices, valid_mask,
           max_prototypes=None, **_ignored):
    similarities = np.asarray(similarities, dtype=np.float32)
    labels = np.asarray(labels)
    prototypes = np.asarray(prototypes, dtype=np.float32)
    proto_indices = np.asarray(proto_indices)
    valid_mask = np.asarray(valid_mask).astype(bool)

    starts = proto_indices[:, 0].astype(np.int64)
    ends = proto_indices[:, 1].astype(np.int64)
    counts = ends - starts
    simple = (similarities.shape == (B, C, P) and prototypes.shape == (T, D)
              and bool((starts == np.arange(C) * P).all())
              and bool((counts == P).all()) and bool(valid_mask.all()))
    if not simple:
        return _numpy_fallback(similarities, labels, prototypes,
                               proto_indices, valid_mask)

    labels_i = labels.astype(np.int64)
    # host prep: dtype/layout only
    norm = np.sqrt((prototypes * prototypes).sum(-1, keepdims=True))
    pn = (prototypes / np.maximum(norm, 1e-12)).astype(np.float32)
    sims16 = similarities.astype(np.float16)
    proto_class = np.arange(T) // P

    in_maps = []
    for c in range(NCORES):
        s3 = sims16[c * BC:(c + 1) * BC].reshape(NT, 128, C, P)
        s4 = s3.transpose(3, 0, 1, 2)              # [P, tile, part, C]
        blocks = []
        for g, gsz in enumerate(GROUPS):
            blk = s4[:, GOFF[g]:GOFF[g] + gsz]     # [P, gsz, 128, C]
            blocks.append(blk.transpose(2, 0, 1, 3).reshape(128, P * gsz * C))
        sg = np.ascontiguousarray(np.concatenate(blocks, axis=1))
        lab16 = labels_i[c * BC:(c + 1) * BC].reshape(NT, 128).T.astype(np.float32)
        r0 = c * TRV
        pnr = np.concatenate([pn[r0:], pn[:r0]], axis=0)  # rotated rows
        pT = np.ascontiguousarray(pnr.T.astype(np.float16))  # [D, T]
        # band mask: local col j of band slot k
        rows = np.arange(128)
        jcols = np.concatenate([np.arange(BAND0), np.arange(991, 1000)])
        gr = (r0 + rows) % T                      # global row
        gc = (r0 + jcols[None, :]) % T            # global col per band slot
        bm = (proto_class[gr][:, None] == proto_class[gc]).astype(np.float16)
        bm[rows[:, None] == jcols[None, :]] = 0.0  # exclude self
        bm[TRV:] = 0.0
        in_maps.append(dict(sims=sg, lab=lab16, pt0=pT[0:128], pt1=pT[128:256],
                            bandm=bm))

    nc = _get_program()
    res = run_bass_kernel_spmd(nc, in_maps, core_ids=list(range(NCORES)))
    results = res.results

    f32 = np.float32
    cls = np.sum(np.stack([results[c]["out_cls"] for c in range(NCORES)]),
                 axis=0, dtype=np.float32)  # [2, C]
    ownr = np.sum(np.stack([np.diagonal(results[c]["out_own"])
                            for c in range(NCORES)]),
                  axis=0, dtype=np.float32)  # [C]
    sep_cls_sum = cls[0] * f32(-0.25)
    cls_n = cls[1] * f32(-0.25)
    A_c = ownr * f32(-0.25)          # sum of own_smax per class
    own_sum = cls_n - A_c            # sum of own_min = sum(1 - own_smax)

    has = cls_n > 0.5
    nvalid = f32(max(int(has.sum()), 1))
    mean_c = (own_sum / np.maximum(cls_n, f32(1.0))).astype(f32)
    w = (f32(1.0) / np.sqrt(cls_n + f32(1e-6))).astype(f32)
    cluster = f32(np.where(has, w * mean_c, f32(0.0)).sum(dtype=np.float32)
                  / nvalid * f32(CLST_SCALE))
    sep = f32(np.where(has, sep_cls_sum / np.maximum(cls_n, f32(1.0)), f32(0.0))
              .sum(dtype=np.float32) / nvalid * f32(SEP_SCALE))

    divrow = np.concatenate([results[c]["out_pr"][:TRV, 0] for c in range(NCORES)])
    conrow = np.concatenate([
        results[c]["out_pr"][:TRV, 1] + results[c]["out_pr"][:TRV, 2]
        for c in range(NCORES)])
    # subtract the gram diagonal, reproduced from the same fp16 data
    pn16f = pn.astype(np.float16).astype(np.float32)
    diagsum = f32((pn16f * pn16f).sum(dtype=np.float32))

    cls_pair = np.zeros(C, np.float32)
    np.add.at(cls_pair, proto_class, divrow)
    npairs = (counts * (counts - 1)).astype(np.float32)
    dvalid = counts > 1
    ndv = f32(max(int(dvalid.sum()), 1))
    div = f32(np.where(dvalid, cls_pair / np.maximum(npairs, f32(1.0)), f32(0.0))
              .sum(dtype=np.float32) / ndv * f32(DIV_SCALE))

    nvp = f32(T * T - T)
    contrast = f32((conrow.sum(dtype=np.float32) - diagsum) / nvp
                   * f32(CONTRASTIVE_SCALE))

    total = f32(cluster + sep + div + contrast)
    return np.array([cluster, sep, div, contrast, total], dtype=np.float32)


# revision 46
# speedup vs baseline: 1.0105x; 1.0105x over previous
"""BalancedPrototypeLoss on 8 Trainium2 NeuronCores.

Strategy (data-parallel over batch, row-parallel over prototypes):
  - similarities [16384,100,10] sharded along batch across 8 cores
    (2048 samples/core = 16 tiles of 128 partitions), streamed in 4
    groups of 4 tiles with the P-dim split 5+5: the second half is
    DMA'd with a SWDGE accumulate-max onto the first (the DMA engine
    does tree level 1), the rest of the max-over-P tree is 3 dense
    fp16 tensor_tensor max ops (DVE 2x mode).
  - one-hot(-4x) generated on device from labels via iota+tensor_scalar.
  - per-tile own/other class stats via tensor_tensor_reduce
    (own = dot(smax, -4*onehot) row-sum; other = max(smax - 4*onehot)),
    then per-tile PE matmuls RHS^T @ (-4*onehot) accumulated in PSUM
    giving per-class partials [3,100].
  - prototypes are normalized + transposed + rotated by 125*core on the
    host (O(T*D) prep); gram rows for this core's 125-row slice via two
    PE matmuls into PSUM; diversity needs only a 146-col band around
    the diagonal (classes are 10-wide contiguous), contrastive row sums
    via scalar-engine copy-accumulate, diagonal via identity mask.
  - host combines the tiny per-core partials ([3,100] + [128,4] each)
    and evaluates the final scalar formulas in float32.
"""

import sys

_TRN_REPO = "/opt/trn_rl_repo"
if _TRN_REPO not in sys.path:
    sys.path.insert(0, _TRN_REPO)

import numpy as np

import concourse.bacc as bacc
import concourse.mybir as mybir
from concourse import tile
from concourse.masks import make_identity
from concourse.bass_utils import run_bass_kernel_spmd

fp32 = mybir.dt.float32
fp16 = mybir.dt.float16
Alu = mybir.AluOpType
Act = mybir.ActivationFunctionType
Axis = mybir.AxisListType

B, C, P, D, T = 16384, 100, 10, 256, 1000
NCORES = 8
BC = B // NCORES     # 2048 samples per core
NT = BC // 128       # 16 batch tiles per core
GRP = 4              # tiles per group
NG = NT // GRP       # 4 groups
TRV = T // NCORES    # 125 prototype rows per core
BAND0 = 137          # gram band: local cols [0,137)
BAND1 = 9            # plus wraparound cols [991,1000)
BANDW = BAND0 + BAND1
MARGIN = 0.3
CLST_SCALE = 0.8
SEP_SCALE = 0.08
DIV_SCALE = 0.01
CONTRASTIVE_SCALE = 0.1

USE_DMA_L1 = False   # neuronx-cc rejects DMA accum max (add-only CCE)
GROUPS = [2, 4, 4, 4, 2]          # batch tiles per streamed group
GOFF = [sum(GROUPS[:i]) for i in range(len(GROUPS))]
JO_ON_GPSIMD = False              # Q7 TT also slower + port contention
OH_ON_GPSIMD = False              # Q7 tensor_scalar is ~7x slower than DVE

_PROGRAMS = {}


def _build(dma_l1: bool):
    nc = bacc.Bacc("TRN2", target_bir_lowering=False, debug=False,
                   num_devices=1)
    sims_d = nc.dram_tensor("sims", [128, P * NT * C], fp16,
                            kind="ExternalInput").ap()
    lab_d = nc.dram_tensor("lab", [128, NT], fp32, kind="ExternalInput").ap()
    pt0_d = nc.dram_tensor("pt0", [128, T], fp16, kind="ExternalInput").ap()
    pt1_d = nc.dram_tensor("pt1", [128, T], fp16, kind="ExternalInput").ap()
    bandm_d = nc.dram_tensor("bandm", [128, BANDW], fp16,
                             kind="ExternalInput").ap()
    outcls_d = nc.dram_tensor("out_cls", [2, C], fp32, kind="ExternalOutput").ap()
    outown_d = nc.dram_tensor("out_own", [C, C], fp32, kind="ExternalOutput").ap()
    outpr_d = nc.dram_tensor("out_pr", [128, 4], fp32, kind="ExternalOutput").ap()

    with tile.TileContext(nc) as tc:
        with (
            tc.tile_pool(name="consts", bufs=1) as consts,
            tc.tile_pool(name="simin", bufs=2) as simin,
            tc.tile_pool(name="tree", bufs=2) as treep,
            tc.tile_pool(name="junk", bufs=2) as junkp,
            tc.tile_pool(name="outp", bufs=1) as outp,
            tc.tile_pool(name="psA", bufs=1, space="PSUM") as psA,
            tc.tile_pool(name="psG", bufs=2, space="PSUM") as psG,
        ):
            # ---- hoisted loads: first sims group leads, consts interleave ----
            sAs = []
            for g, gsz in enumerate(GROUPS):
                sAs.append(consts.tile([128, P, gsz, C], fp16, name=f"sA{g}",
                                       tag=f"sA{g}"))
            lab = consts.tile([128, NT], fp32, tag="lab")
            pt0 = consts.tile([128, T], fp16, tag="pt0")
            pt1 = consts.tile([128, T], fp16, tag="pt1")
            bandm = consts.tile([128, BANDW], fp16, tag="bandm")

            def sims_dma(g, eng):
                off = P * GOFF[g] * C
                sz = P * GROUPS[g] * C
                eng.dma_start(sAs[g][:], sims_d[:, off:off + sz])

            nc.sync.dma_start(lab[:], lab_d[:])
            sims_dma(0, nc.scalar)
            for g in range(1, len(GROUPS)):
                sims_dma(g, nc.sync)
            nc.sync.dma_start(pt0[:], pt0_d[:])
            nc.sync.dma_start(pt1[:], pt1_d[:])
            nc.sync.dma_start(bandm[:], bandm_d[:])

            iot = consts.tile([128, C], fp16, tag="iot")
            nc.gpsimd.iota(iot[:], pattern=[[1, C]], base=0,
                           channel_multiplier=0,
                           allow_small_or_imprecise_dtypes=True)
            nhalf = consts.tile([128, 1], fp32, tag="nhalf")
            nc.vector.memset(nhalf[:], -0.5)

            # one-hot * (-4) for every tile, on device from labels
            oh_eng = nc.gpsimd if OH_ON_GPSIMD else nc.vector
            OH = consts.tile([128, NT, C], fp16, tag="OH")
            with tc.high_priority():
                for i in range(NT):
                    oh_eng.tensor_scalar(OH[:, i, :], iot[:], lab[:, i:i + 1],
                                         -4.0, op0=Alu.is_equal, op1=Alu.mult)

            RHS = consts.tile([128, 2, NT], fp16, tag="RHS")
            nc.vector.memset(RHS[:, 1, :], 1.0)
            maxr = consts.tile([128, NT], fp32, tag="maxr")

            # ---- gram: rows 0:128 (this core's rotated slice) ----
            g01 = []
            for nh in range(2):
                g = psG.tile([128, 500], fp32, name=f"g{nh}", tag=f"g{nh}")
                nc.tensor.matmul(g[:], pt0[:, 0:128], pt0[:, 500 * nh:500 * (nh + 1)],
                                 start=True, stop=False)
                nc.tensor.matmul(g[:], pt1[:, 0:128], pt1[:, 500 * nh:500 * (nh + 1)],
                                 start=False, stop=True)
                g01.append(g)

            opr = outp.tile([128, 4], fp32, tag="opr")
            # diversity band: relu(g - 0.5) over local cols [0,137) u [991,1000)
            bandr = junkp.tile([128, BANDW], fp16, tag="bandr")
            nc.scalar.activation(bandr[:, 0:BAND0], g01[0][:, 0:BAND0],
                                 Act.Relu, bias=nhalf[:])
            nc.scalar.activation(bandr[:, BAND0:BANDW], g01[1][:, 491:500],
                                 Act.Relu, bias=nhalf[:])
            bandj = junkp.tile([128, BANDW], fp32, tag="bandj")
            nc.vector.tensor_tensor(bandj[:], bandr[:], bandm[:], op=Alu.mult)
            nc.vector.tensor_reduce(opr[:, 0:1], bandj[:], axis=Axis.X, op=Alu.add)
            # contrastive: full row sums (scalar engine accumulate)
            for nh in range(2):
                junkc = junkp.tile([128, 500], fp32, name=f"jc{nh}", tag="jc")
                nc.scalar.activation(junkc[:], g01[nh][:], Act.Copy,
                                     accum_out=opr[:, 1 + nh:2 + nh])
            # (diagonal self-similarity is reproduced on the host from pn16)
            nc.vector.memset(opr[:, 3:4], 0.0)
            nc.sync.dma_start(outpr_d[:], opr[:])

            # ---- batch stream ----
            cls_ps = psA.tile([2, C], fp32, tag="cls")
            own_ps = psA.tile([C, C], fp32, tag="own")
            for g, gsz in enumerate(GROUPS):
                t0 = GOFF[g]
                sl = slice(t0, t0 + gsz)
                last = (g == len(GROUPS) - 1)
                sA = sAs[g]
                t1 = treep.tile([128, 5, gsz, C], fp16, name=f"t1_{g}", tag="t1")
                nc.vector.tensor_tensor(t1[:], sA[:, 0:5], sA[:, 5:10],
                                        op=Alu.max)
                t2 = treep.tile([128, 2, gsz, C], fp16, name=f"t2_{g}", tag="t2")
                nc.vector.tensor_tensor(t2[:], t1[:, 0:2], t1[:, 2:4],
                                        op=Alu.max)
                t3 = treep.tile([128, 1, gsz, C], fp16, name=f"t3_{g}", tag="t3")
                nc.vector.tensor_tensor(t3[:], t2[:, 0:1], t2[:, 1:2], op=Alu.max)
                SM = treep.tile([128, gsz, C], fp16, name=f"SM{g}", tag="SM")
                nc.vector.tensor_tensor(SM[:], t3[:, 0], t1[:, 4], op=Alu.max)

                js = junkp.tile([128, gsz, C], fp16, name=f"js{g}", tag="js")
                # own: diag(OH^T @ SM) on the PE gives -4*per-class own sums
                for t in range(gsz):
                    i = t0 + t
                    nc.tensor.matmul(own_ps[:], OH[:, i, :], SM[:, t, :],
                                     start=(i == 0), stop=(i == NT - 1))
                # other: max(smax - 4*onehot)
                nc.vector.tensor_tensor(js[:], SM[:], OH[:, sl, :], op=Alu.add)
                nc.vector.tensor_reduce(maxr[:, sl], js[:], axis=Axis.X,
                                        op=Alu.max)
                # RHS rows: relu(other_smax - (1-margin)) ; 1
                nc.vector.tensor_scalar(RHS[:, 0, sl], maxr[:, sl],
                                        -(1.0 - MARGIN), 0.0,
                                        op0=Alu.add, op1=Alu.max)
                for t in range(gsz):
                    i = t0 + t
                    nc.tensor.matmul(cls_ps[:], RHS[:, :, i], OH[:, i, :],
                                     start=(i == 0), stop=(i == NT - 1))

            ocl = outp.tile([2, C], fp32, tag="ocl")
            nc.vector.tensor_copy(ocl[:], cls_ps[:])
            nc.sync.dma_start(outcls_d[:], ocl[:])
            oow = outp.tile([C, C], fp32, tag="oow")
            nc.vector.tensor_copy(oow[:], own_ps[:])
            nc.sync.dma_start(outown_d[:], oow[:])

    nc.compile()
    return nc


def _get_program():
    key = bool(USE_DMA_L1)
    if key not in _PROGRAMS:
        _PROGRAMS[key] = _build(key)
    return _PROGRAMS[key]


def _numpy_fallback(similarities, labels, prototypes, proto_indices, valid_mask):
    """Pure-numpy replication of the reference (for unexpected shapes)."""
    s = similarities.astype(np.float64)
    Bx, Cx, Px = s.shape
    Tx = prototypes.shape[0]
    distances = 1.0 - s
    starts = proto_indices[:, 0]
    ends = proto_indices[:, 1]
    counts = ends - starts
    pvalid = np.arange(Px)[None, :] < counts[:, None]
    dmask = np.where(pvalid[None, :, :], distances, np.inf)
    min_all = dmask.min(axis=-1)
    own_min = min_all[np.arange(Bx), labels]
    cls_n = np.bincount(labels, minlength=Cx).astype(np.float64)
    cls_sum = np.bincount(labels, weights=own_min, minlength=Cx)
    has = cls_n > 0
    nvalid = max(int(has.sum()), 1)
    mean_c = cls_sum / np.maximum(cls_n, 1.0)
    w = 1.0 / np.sqrt(cls_n + 1e-6)
    cluster = np.where(has, w * mean_c, 0.0).sum() / nvalid * CLST_SCALE
    m2 = min_all.copy()
    m2[np.arange(Bx), labels] = np.inf
    other_min = m2.min(axis=-1)
    sep_term = np.maximum(MARGIN - other_min, 0.0)
    sep_cls = np.bincount(labels, weights=sep_term, minlength=Cx)
    sep = np.where(has, sep_cls / np.maximum(cls_n, 1.0), 0.0).sum() / nvalid * SEP_SCALE
    pr = prototypes.astype(np.float64)
    norm = np.sqrt((pr * pr).sum(-1, keepdims=True))
    pn = pr / np.maximum(norm, 1e-12)
    sim = pn @ pn.T
    proto_class = np.searchsorted(starts, np.arange(Tx), side="right") - 1
    same = proto_class[:, None] == proto_class[None, :]
    offd = ~np.eye(Tx, dtype=bool)
    pair = same & offd
    relv = np.maximum(sim - 0.5, 0.0)
    row_sum = np.where(pair, relv, 0.0).sum(1)
    cls_pair = np.bincount(proto_class, weights=row_sum, minlength=Cx)
    npairs = (counts * (counts - 1)).astype(np.float64)
    dvalid = counts > 1
    ndv = max(int(dvalid.sum()), 1)
    div = np.where(dvalid, cls_pair / np.maximum(npairs, 1.0), 0.0).sum() / ndv * DIV_SCALE
    vm = valid_mask.astype(bool)
    vpair = (vm[:, None] & vm[None, :]) & offd
    nvp = max(int(vpair.sum()), 1)
    contrast = np.where(vpair, sim, 0.0).sum() / nvp * CONTRASTIVE_SCALE
    total = cluster + sep + div + contrast
    return np.array([cluster, sep, div, contrast, total], dtype=np.float32)


def kernel(similarities, labels, prototypes, proto_indices, valid_mask,
           max_prototypes=None, **_ignored):
    similarities = np.asarray(similarities, dtype=np.float32)
    labels = np.asarray(labels)
    prototypes = np.asarray(prototypes, dtype=np.float32)
    proto_indices = np.asarray(proto_indices)
    valid_mask = np.asarray(valid_mask).astype(bool)

    starts = proto_indices[:, 0].astype(np.int64)
    ends = proto_indices[:, 1].astype(np.int64)
    counts = ends - starts
    simple = (similarities.shape == (B, C, P) and prototypes.shape == (T, D)
              and bool((starts == np.arange(C) * P).all())
              and bool((counts == P).all()) and bool(valid_mask.all()))
    if not simple:
        return _numpy_fallback(similarities, labels, prototypes,
                               proto_indices, valid_mask)

    labels_i = labels.astype(np.int64)
    # host prep: dtype/layout only
    norm = np.sqrt((prototypes * prototypes).sum(-1, keepdims=True))
    pn = (prototypes / np.maximum(norm, 1e-12)).astype(np.float32)
    sims16 = similarities.astype(np.float16)
    proto_class = np.arange(T) // P

    in_maps = []
    for c in range(NCORES):
        s3 = sims16[c * BC:(c + 1) * BC].reshape(NT, 128, C, P)
        s4 = s3.transpose(3, 0, 1, 2)              # [P, tile, part, C]
        blocks = []
        for g, gsz in enumerate(GROUPS):
            blk = s4[:, GOFF[g]:GOFF[g] + gsz]     # [P, gsz, 128, C]
            blocks.append(blk.transpose(2, 0, 1, 3).reshape(128, P * gsz * C))
        sg = np.ascontiguousarray(np.concatenate(blocks, axis=1))
        lab16 = labels_i[c * BC:(c + 1) * BC].reshape(NT, 128).T.astype(np.float32)
        r0 = c * TRV
        pnr = np.concatenate([pn[r0:], pn[:r0]], axis=0)  # rotated rows
        pT = np.ascontiguousarray(pnr.T.astype(np.float16))  # [D, T]
        # band mask: local col j of band slot k
        rows = np.arange(128)
        jcols = np.concatenate([np.arange(BAND0), np.arange(991, 1000)])
        gr = (r0 + rows) % T                      # global row
        gc = (r0 + jcols[None, :]) % T            # global col per band slot
        bm = (proto_class[gr][:, None] == proto_class[gc]).astype(np.float16)
        bm[rows[:, None] == jcols[None, :]] = 0.0  # exclude self
        bm[TRV:] = 0.0
        in_maps.append(dict(sims=sg, lab=lab16, pt0=pT[0:128], pt1=pT[128:256],
                            bandm=bm))

    nc = _get_program()
    res = run_bass_kernel_spmd(nc, in_maps, core_ids=list(range(NCORES)))
    results = res.results

    f32 = np.float32
    cls = np.sum(np.stack([results[c]["out_cls"] for c in range(NCORES)]),
                 axis=0, dtype=np.float32)  # [2, C]
    ownr = np.sum(np.stack([np.diagonal(results[c]["out_own"])
                            for c in range(NCORES)]),
                  axis=0, dtype=np.float32)  # [C]
    sep_cls_sum = cls[0] * f32(-0.25)
    cls_n = cls[1] * f32(-0.25)
    A_c = ownr * f32(-0.25)          # sum of own_smax per class
    own_sum = cls_n - A_c            # sum of own_min = sum(1 - own_smax)

    has = cls_n > 0.5
    nvalid = f32(max(int(has.sum()), 1))
    mean_c = (own_sum / np.maximum(cls_n, f32(1.0))).astype(f32)
    w = (f32(1.0) / np.sqrt(cls_n + f32(1e-6))).astype(f32)
    cluster = f32(np.where(has, w * mean_c, f32(0.0)).sum(dtype=np.float32)
                  / nvalid * f32(CLST_SCALE))
    sep = f32(np.where(has, sep_cls_sum / np.maximum(cls_n, f32(1.0)), f32(0.0))
              .sum(dtype=np.float32) / nvalid * f32(SEP_SCALE))

    divrow = np.concatenate([results[c]["out_pr"][:TRV, 0] for c in range(NCORES)])
    conrow = np.concatenate([
        results[c]["out_pr"][:TRV, 1] + results[c]["out_pr"][:TRV, 2]
        for c in range(NCORES)])
    # subtract the gram diagonal, reproduced from the same fp16 data
    pn16f = pn.astype(np.float16).astype(np.float32)
    diagsum = f32((pn16f * pn16f).sum(dtype=np.float32))

    cls_pair = np.zeros(C, np.float32)
    np.add.at(cls_pair, proto_class, divrow)
    npairs = (counts * (counts - 1)).astype(np.float32)
    dvalid = counts > 1
    ndv = f32(max(int(dvalid.sum()), 1))
    div = f32(np.where(dvalid, cls_pair / np.maximum(npairs, f32(1.0)), f32(0.0))
              .sum(dtype=np.float32) / ndv * f32(DIV_SCALE))

    nvp = f32(T * T - T)
    contrast = f32((conrow.sum(dtype=np.float32) - diagsum) / nvp
                   * f32(CONTRASTIVE_SCALE))

    total = f32(cluster + sep + div + contrast)
    return np.array([cluster, sep, div, contrast, total], dtype=np.float32)
